# revision 1
# baseline (speedup 1.0000x reference)
"""Circulant matmul kernel for Trainium2 (8 NeuronCores, SPMD).

Problem: out = input @ K + bias, where K[c, n] = weight[(c - n) mod 4096],
input is [1024, 4096] f32, weight/bias are [4096] f32.

Strategy (tensor-parallel / column-shard, per the sharding hint):
  - Host materializes X^T in bf16 (replicated to all 8 cores) and each
    core's 512-column slice of the circulant matrix K in bf16.
  - Core c computes out[:, 512c:512(c+1)] = X @ K_c + bias_c in fp32 PSUM.
    No collectives; host concatenates the 8 column slices.

Device kernel structure (per core):
  - xt chunks (32 x [128, 1024] bf16) DMA'd on the sync HWDGE queue,
    kc chunks (32 x [128, 512] bf16) on the scalar HWDGE queue, so
    descriptor generation is parallelized across both HW-DGE rings.
  - PE warm-up: full-width dummy matmuls on a DVE-memset scratch tile
    while the first input chunks land (lifts the HAM clock gate early).
  - Phase 1 processes chunks 0..23 across all 8 batch tiles (co-major,
    matches DMA arrival); phase 2 finishes each batch tile in turn
    (bt-major) so the bias-add + output DMA epilogues overlap the
    remaining matmuls.
"""

import numpy as np
import ml_dtypes

import concourse.bass as bass
import concourse.mybir as mybir
import concourse.tile as tile
from concourse import bacc
from concourse.bass import ts
from concourse.bass_utils import run_bass_kernel_spmd

N = 4096
BATCH = 1024
NCORES = 8
NSHARD = N // NCORES          # 512 output columns per core
P = 128                       # partitions
CO = N // P                   # 32 contraction chunks
BT = BATCH // P               # 8 batch tiles
CO_PH1 = CO - BT              # chunks processed co-major in phase 1

N_WARMUP = 9                  # full-width dummy matmuls to lift the HAM clock gate

BF16 = mybir.dt.bfloat16
F32 = mybir.dt.float32


def build_nc():
    """Build the per-core Bass program (same program on all cores; data differs)."""
    nc = bacc.Bacc("TRN2", target_bir_lowering=False, debug=False)

    xt_d = nc.dram_tensor("xt", [N, BATCH], BF16, kind="ExternalInput").ap()
    kc_d = nc.dram_tensor("kc", [N, NSHARD], BF16, kind="ExternalInput").ap()
    bias_d = nc.dram_tensor("biasb", [P, NSHARD], F32, kind="ExternalInput").ap()
    out_d = nc.dram_tensor("out", [BATCH, NSHARD], BF16, kind="ExternalOutput").ap()

    xt_r = xt_d.rearrange("(co ci) b -> ci co b", ci=P)      # [128, 32, 1024]
    kc_r = kc_d.rearrange("(co ci) n -> ci co n", ci=P)      # [128, 32, 512]

    with tile.TileContext(nc) as tc:
        with (
            tc.tile_pool(name="xpool", bufs=CO) as xpool,
            tc.tile_pool(name="kpool", bufs=CO) as kpool,
            tc.tile_pool(name="cpool", bufs=1) as cpool,
            tc.tile_pool(name="opool", bufs=4) as opool,
            tc.tile_pool(name="psum", bufs=BT, space="PSUM") as psum_pool,
        ):
            # scratch for PE warm-up, memset on the vector engine (fast start)
            scratch = cpool.tile([P, NSHARD], BF16, tag="scratch")
            nc.vector.memset(scratch[:], 0.125)

            # input streams: kc on scalar ring, xt on sync ring
            xt_tiles = []
            kc_tiles = []
            for co in range(CO):
                ktt = kpool.tile([P, NSHARD], BF16, tag="kc")
                nc.scalar.dma_start(ktt[:], kc_r[:, co, :])
                kc_tiles.append(ktt)
                xtt = xpool.tile([P, BATCH], BF16, tag="xt")
                nc.sync.dma_start(xtt[:], xt_r[:, co, :])
                xt_tiles.append(xtt)
            # bias last on the scalar ring: only needed for the epilogues
            bias_sb = cpool.tile([P, NSHARD], F32, tag="bias")
            nc.scalar.dma_start(bias_sb[:], bias_d)

            psum_tiles = [
                psum_pool.tile([P, NSHARD], F32, tag="ps", name=f"ps{bt}")
                for bt in range(BT)
            ]

            # PE warm-up: full 128-row dummy matmuls on scratch (HAM needs
            # real array activity; results are discarded by start=True below)
            for i in range(N_WARMUP):
                nc.tensor.matmul(
                    psum_tiles[i % BT][:],
                    scratch[:, :P],
                    scratch[:],
                    start=True,
                    stop=True,
                )

            # phase 1: chunks 0..CO_PH1-1, co-major (matches DMA arrival order)
            for co in range(CO_PH1):
                for bt in range(BT):
                    nc.tensor.matmul(
                        psum_tiles[bt][:],
                        xt_tiles[co][:, ts(bt, P)],   # lhsT [c=128, b=128]
                        kc_tiles[co][:],              # rhs  [c=128, n=512]
                        start=(co == 0),
                        stop=False,
                    )

            # phase 2: finish batch tiles one at a time; epilogue overlaps MMs
            for bt in range(BT):
                for co in range(CO_PH1, CO):
                    nc.tensor.matmul(
                        psum_tiles[bt][:],
                        xt_tiles[co][:, ts(bt, P)],
                        kc_tiles[co][:],
                        start=False,
                        stop=(co == CO - 1),
                    )
                out_sb = opool.tile([P, NSHARD], BF16, tag="osb")
                nc.vector.tensor_add(out_sb[:], psum_tiles[bt][:], bias_sb[:])
                nc.sync.dma_start(out_d[ts(bt, P), :], out_sb[:])

    nc.compile()
    return nc


def prepare_in_maps(input, weight, bias):
    x = np.asarray(input, dtype=np.float32)
    w = np.asarray(weight, dtype=np.float32)
    b = np.asarray(bias, dtype=np.float32)

    xt = np.ascontiguousarray(x.T).astype(ml_dtypes.bfloat16)   # [4096, 1024]

    c = np.arange(N)
    in_maps = []
    for core in range(NCORES):
        n0 = core * NSHARD
        idx = (c[:, None] - (n0 + np.arange(NSHARD))[None, :]) % N
        kc = w[idx].astype(ml_dtypes.bfloat16)                  # [4096, 512]
        bias_tile = np.ascontiguousarray(
            np.broadcast_to(b[n0 : n0 + NSHARD].astype(np.float32), (P, NSHARD))
        )
        in_maps.append({"xt": xt, "kc": kc, "biasb": bias_tile})
    return in_maps


_NC_CACHE = None


def _get_nc():
    global _NC_CACHE
    if _NC_CACHE is None:
        _NC_CACHE = build_nc()
    return _NC_CACHE


def kernel(**inputs):
    nc = _get_nc()
    in_maps = prepare_in_maps(inputs["input"], inputs["weight"], inputs["bias"])
    res = run_bass_kernel_spmd(nc, in_maps, list(range(NCORES)))
    out = np.empty((BATCH, N), dtype=np.float32)
    for core in range(NCORES):
        out[:, core * NSHARD : (core + 1) * NSHARD] = res.results[core]["out"].astype(
            np.float32
        )
    return out



# revision 2
# speedup vs baseline: 2.3903x; 2.3903x over previous
"""Circulant matmul kernel for Trainium2 (8 NeuronCores, SPMD).

Problem: out = input @ K + bias, where K[c, n] = weight[(c - n) mod 4096],
input is [1024, 4096] f32, weight/bias are [4096] f32.

Strategy: the circulant matmul is a cyclic convolution, so it CRT-decomposes
over the factorization of z^4096 - 1 (Bruun tree, all real):

  z^4096-1 -> (z^2048-1)(z^2048+1) -> ... -> 8 coprime degree-512 moduli
  (z^512-1, z^512+1, and z^512 +/- a*z^256 + 1 for a in sqrt(2-+c) chains)

Each core computes ONE residue product y_p = (x mod m_p) @ M_p, a
[1024,512] @ [512,512] matmul in bf16 with fp32 PSUM accumulation -- 1/8th
the FLOPs of the dense circulant matmul. The host (sharding/gather stage)
applies the forward reductions to x / builds the M_p from weight, and
reconstructs out = CRT^-1(y_0..y_7) + bias. All O(n^2)-heavy work stays on
device; host work is O(batch*n) folds, same class as input transposition.

Device kernel (per core): 4 contraction chunks x 8 batch tiles = 32
matmuls of N=512 into 8 PSUM banks; a few warm-up matmuls lift the HAM
clock gate while the first DMA chunks land; phase 2 finishes batch tiles
one at a time so the PSUM->SBUF bf16 copies (alternating DVE / ACT
engines) and output DMAs overlap the remaining matmuls.
"""

import numpy as np
import ml_dtypes

import concourse.bass as bass
import concourse.mybir as mybir
import concourse.tile as tile
from concourse import bacc
from concourse.bass_utils import run_bass_kernel_spmd

N = 4096
BATCH = 1024
NCORES = 8
DEPTH = 3
L = N >> DEPTH                # 512: leaf modulus degree = per-core contraction
P = 128                       # partitions
KCH = L // P                  # 4 contraction chunks
BT = BATCH // P               # 8 batch tiles
N_WARM = 5                    # dummy matmuls to lift the HAM clock gate

BF16 = mybir.dt.bfloat16
F32 = mybir.dt.float32


# ---------- CRT tree (host side) ----------
# modulus encoding: ("cyc", n) = z^n - 1 ; ("f2", n, c) = z^n + c*z^(n/2) + 1

def _children(mod):
    if mod[0] == "cyc":
        n = mod[1]
        return [("cyc", n // 2), ("f2", n // 2, 0.0)]
    _, n, c = mod
    a = np.sqrt(2.0 - c)
    return [("f2", n // 2, a), ("f2", n // 2, -a)]


def _reduce_mod(p, mod):
    """p [..., W] -> p mod `mod` [..., n]."""
    if mod[0] == "cyc":
        n = mod[1]
        while p.shape[-1] > n:
            lo, hi = p[..., :n], p[..., n : 2 * n]
            rest = p[..., 2 * n :]
            lo = lo.copy()
            lo[..., : hi.shape[-1]] += hi
            p = np.concatenate([lo, rest], axis=-1)
        return p
    _, n, c = mod
    q = n // 2
    while p.shape[-1] > n:
        lo, hi = p[..., :n], p[..., n:]
        W = max(n, q + hi.shape[-1])
        out = np.zeros(p.shape[:-1] + (W,), dtype=p.dtype)
        out[..., :n] = lo
        out[..., : hi.shape[-1]] -= hi
        out[..., q : q + hi.shape[-1]] -= c * hi
        p = out
    return p


def _mod_levels():
    lv = [[("cyc", N)]]
    for _ in range(DEPTH):
        lv.append([ch for m in lv[-1] for ch in _children(m)])
    return lv


_LEVELS = _mod_levels()
LEAVES = _LEVELS[DEPTH]


def _recon(y1, y2, parent):
    """Inverse CRT step: y1 = p mod m1, y2 = p mod m2 -> p mod parent."""
    if parent[0] == "cyc":
        return np.concatenate([(y1 + y2) * 0.5, (y1 - y2) * 0.5], axis=-1)
    _, n, c = parent
    h = n // 2
    q = h // 2
    a = np.sqrt(2.0 - c)
    s = (y1 + y2) * 0.5
    d = (y1 - y2) * 0.5
    W = 3 * h - q
    p = np.zeros(s.shape[:-1] + (W,), dtype=s.dtype)
    p[..., :h] += s
    p[..., 2 * h - q : 3 * h - q] += d / a
    p[..., h - q : 2 * h - q] += d / a
    p[..., q : q + h] -= a * d
    return _reduce_mod(p, parent)


def _tree_recon(ys):
    cur = list(ys)
    for lvl in range(DEPTH, 0, -1):
        parents = _LEVELS[lvl - 1]
        cur = [_recon(cur[2 * i], cur[2 * i + 1], parents[i]) for i in range(len(parents))]
    return cur[0]


def _mulmat(vred, mod):
    """M[r, k] = coeff of z^k in (z^r * vred(z)) mod `mod`."""
    n = mod[1]
    M = np.zeros((n, n))
    row = vred.astype(np.float64).copy()
    for r in range(n):
        M[r] = row
        top = row[-1]
        row = np.roll(row, 1)
        row[0] = 0.0
        if mod[0] == "cyc":
            row[0] += top
        else:
            row[0] -= top
            row[n // 2] -= mod[2] * top
    return M


def _prechunk(a, kch):
    """[kch*128, F] -> [128, kch*F] with free layout (k, f)."""
    f = a.shape[1]
    return np.ascontiguousarray(
        a.reshape(kch, P, f).transpose(1, 0, 2).reshape(P, kch * f)
    )


# ---------- device program ----------

def build_nc():
    nc = bacc.Bacc("TRN2", target_bir_lowering=False, debug=False)

    xt_d = nc.dram_tensor("xt", [P, KCH * BATCH], BF16, kind="ExternalInput").ap()
    mp_d = nc.dram_tensor("mp", [P, KCH * L], BF16, kind="ExternalInput").ap()
    y_d = nc.dram_tensor("y", [BATCH, L], BF16, kind="ExternalOutput").ap()

    with tile.TileContext(nc) as tc:
        with (
            tc.tile_pool(name="xpool", bufs=KCH) as xpool,
            tc.tile_pool(name="mpool", bufs=2) as mpool,
            tc.tile_pool(name="cpool", bufs=1) as cpool,
            tc.tile_pool(name="opool", bufs=4) as opool,
            tc.tile_pool(name="psum", bufs=BT, space="PSUM") as psum_pool,
        ):
            scratch = cpool.tile([P, L], BF16, tag="scratch")
            nc.vector.memset(scratch[:], 0.125)

            # x chunks on the sync HWDGE ring, M chunk-pairs on the scalar ring
            x_tiles = []
            for k in range(KCH):
                xt = xpool.tile([P, BATCH], BF16, tag="xt")
                nc.sync.dma_start(xt[:], xt_d[:, k * BATCH : (k + 1) * BATCH])
                x_tiles.append(xt)
            m_tiles = []
            for j in range(KCH // 2):
                mt = mpool.tile([P, 2 * L], BF16, tag="mp")
                nc.scalar.dma_start(mt[:], mp_d[:, j * 2 * L : (j + 1) * 2 * L])
                m_tiles.append(mt)

            psum_tiles = [
                psum_pool.tile([P, L], F32, tag="ps", name=f"ps{bt}")
                for bt in range(BT)
            ]

            def rhs(k):
                return m_tiles[k // 2][:, (k % 2) * L : (k % 2 + 1) * L]

            # PE warm-up (results discarded by phase-1 start=True)
            for i in range(N_WARM):
                nc.tensor.matmul(
                    psum_tiles[i % BT][:],
                    scratch[:, :P],
                    scratch[:],
                    start=True,
                    stop=True,
                )

            # phase 1: contraction chunks 0..KCH-2, k-major (matches DMA arrival)
            for k in range(KCH - 1):
                for bt in range(BT):
                    nc.tensor.matmul(
                        psum_tiles[bt][:],
                        x_tiles[k][:, bt * P : (bt + 1) * P],
                        rhs(k),
                        start=(k == 0),
                        stop=False,
                    )

            # phase 2: finish each batch tile, overlap epilogue with the rest
            k = KCH - 1
            for bt in range(BT):
                nc.tensor.matmul(
                    psum_tiles[bt][:],
                    x_tiles[k][:, bt * P : (bt + 1) * P],
                    rhs(k),
                    start=False,
                    stop=True,
                )
                osb = opool.tile([P, L], BF16, tag="osb")
                if bt % 2 == 0:
                    nc.vector.tensor_copy(osb[:], psum_tiles[bt][:])
                else:
                    nc.scalar.copy(osb[:], psum_tiles[bt][:])
                nc.sync.dma_start(y_d[bt * P : (bt + 1) * P, :], osb[:])

    nc.compile()
    return nc


# ---------- host glue ----------

def prepare_in_maps(input, weight, bias=None):
    x = np.asarray(input, dtype=np.float64)
    w = np.asarray(weight, dtype=np.float64)
    v = w[(-np.arange(N)) % N]  # out = x (cyclic-conv) v

    in_maps = []
    for mod in LEAVES:
        xr = _reduce_mod(x, mod).astype(ml_dtypes.bfloat16)
        vr = _reduce_mod(v[None, :], mod)[0]
        M = _mulmat(vr, mod).astype(ml_dtypes.bfloat16)
        xt = _prechunk(np.ascontiguousarray(xr.T.astype(ml_dtypes.bfloat16)), KCH)
        mp = _prechunk(np.ascontiguousarray(M), KCH)
        in_maps.append({"xt": xt, "mp": mp})
    return in_maps


def assemble(ys, bias):
    """ys: per-core [1024, 512] leaf products -> full [1024, 4096] output."""
    ys = [np.asarray(y).astype(np.float64) for y in ys]
    out = _tree_recon(ys) + np.asarray(bias, dtype=np.float64)
    return out.astype(np.float32)


_NC_CACHE = None


def _get_nc():
    global _NC_CACHE
    if _NC_CACHE is None:
        _NC_CACHE = build_nc()
    return _NC_CACHE


def kernel(**inputs):
    nc = _get_nc()
    in_maps = prepare_in_maps(inputs["input"], inputs["weight"])
    res = run_bass_kernel_spmd(nc, in_maps, list(range(NCORES)))
    return assemble([res.results[c]["y"] for c in range(NCORES)], inputs["bias"])
